# revision 9
# baseline (speedup 1.0000x reference)
"""DirectedEdgeConv (gnn_message_passing) Trainium2 kernel, 8-core SPMD.

out[e] = leaky_relu(edge_attr[e] @ Wself^T + b
                    + T_in[src[e]] + T_out[dst[e]], 0.2)
where T_in  = scatter_mean(edge_attr, dst) @ Win^T + b   [node table]
      T_out = scatter_mean(edge_attr, src) @ Wout^T      [node table]

Sharding / algorithm (v2):
  Core c owns nodes [c*NPC, (c+1)*NPC).  Edges are assigned to cores twice:
  by dst owner (phase A-dst + phase C) and by src owner (phase A-src).

  Phase A-dst (dst-block-grouped edge stream): segment-sum via one-hot
  matmuls -> T_in slice for own nodes (bias folded in) -> ONE AllGather
  (bf16) -> full T_in table on every core.
  Phase A-src: same grouping by src -> T_out slice for own nodes; stays
  LOCAL (phase C only ever needs the core's own T_out rows!).
  Phase C (same dst-block-grouped stream): per 128-edge tile
    psum  = xT_tile.T @ Wself'          (h_self)
    psum += ohT_tile.T @ Tout_block     (T_out[dst] via host-built one-hot)
    y     = Lrelu(psum + gi)            (gi = dma_gather of T_in[src])
  The only per-edge random access left is the T_in gather (bf16 256B rows,
  int16 indices; edges are pre-split lo/hi against two overlapping 32768-row
  table windows so indices fit 15 bits).  Gathers are issued in 4096-index
  chunks to amortize the GpSimd SWDGE emission cost.
"""

import os
import sys

sys.path.insert(0, "/opt/trn_rl_repo")

import numpy as np
import ml_dtypes

BF16NP = ml_dtypes.bfloat16

import concourse.bacc as bacc
import concourse.bass as bass
import concourse.mybir as mybir
import concourse.tile as tile
from concourse import library_config
from concourse.bass_utils import run_bass_kernel_spmd
from concourse.masks import make_identity

P = 128
D = 128
C = 8
HIBASE = 17408   # hi table window starts here; both windows are 32768 rows
LOCAP = 32768
CH = 4096        # gather chunk size (indices per dma_gather call)

F32 = mybir.dt.float32
BF16 = mybir.dt.bfloat16
I16 = mybir.dt.int16

BARRIER = os.environ.get("KBARRIER", "1") == "1"


def _cfg_full():
    return dict(E=600000, N=50000)


def _derive(cfg):
    E, N = cfg["E"], cfg["N"]
    assert N % C == 0
    NPC = N // C
    NB = (NPC + P - 1) // P
    NBP = NB * P
    return NPC, NB, NBP


def build_kernel(cfg, KL, KH, KS):
    """KL/KH: per-dst-block lo/hi tile counts (len NB). KS: per-src-block
    tile counts for phase A-src (len NB). All uniform across cores."""
    E, N = cfg["E"], cfg["N"]
    NPC, NB, NBP = _derive(cfg)
    TROWS = C * NBP

    TOTJ = sum(KL) + sum(KH)      # phase C / A-dst tiles per core
    TOTJS = sum(KS)               # phase A-src tiles per core
    NLO = sum(KL) * P
    NHI = sum(KH) * P
    KMAX = max(max(KL) + max(KH), max(KS))

    # gather chunks: (num_idxs, hi?) list; slot offsets implicit/sequential
    chunks = []
    off = 0
    while off < NLO:
        n = min(CH, NLO - off)
        chunks.append((n, 0))
        off += n
    off = 0
    while off < NHI:
        n = min(CH, NHI - off)
        chunks.append((n, 1))
        off += n
    NCH = len(chunks)

    nc = bacc.Bacc(None, target_bir_lowering=False, debug=False)

    # ---- I/O ----
    agat_d = nc.dram_tensor("agat_d", [P, TOTJ * D], BF16, kind="ExternalInput")
    agat_dt = nc.dram_tensor("agat_dt", [P, TOTJ * D], BF16, kind="ExternalInput")
    ohts = nc.dram_tensor("ohts", [P, TOTJ * P], BF16, kind="ExternalInput")
    va_d = nc.dram_tensor("va_d", [P, TOTJ], F32, kind="ExternalInput")
    agat_s = nc.dram_tensor("agat_s", [P, TOTJS * D], BF16, kind="ExternalInput")
    va_s = nc.dram_tensor("va_s", [P, TOTJS], F32, kind="ExternalInput")
    invc_d = nc.dram_tensor("invc_d", [P, NB], F32, kind="ExternalInput")
    invc_s = nc.dram_tensor("invc_s", [P, NB], F32, kind="ExternalInput")
    gidx = nc.dram_tensor("gidx", [NCH, P, CH // 16], I16, kind="ExternalInput")
    wself = nc.dram_tensor("wself", [D, D], BF16, kind="ExternalInput")
    win = nc.dram_tensor("win", [D, D], BF16, kind="ExternalInput")
    wout = nc.dram_tensor("wout", [D, D], BF16, kind="ExternalInput")
    bbc = nc.dram_tensor("bbc", [P, D], BF16, kind="ExternalInput")
    iota_in = nc.dram_tensor("iota", [P, P], BF16, kind="ExternalInput")
    y = nc.dram_tensor("y", [P, TOTJ * D], BF16, kind="ExternalOutput")

    with tile.TileContext(nc) as tc:
        with (
            tc.tile_pool(name="const", bufs=1) as cpool,
            tc.tile_pool(name="sbuf", bufs=3) as pool,
            tc.tile_pool(name="gpool", bufs=6) as gpool,
            tc.tile_pool(name="small", bufs=4) as spool,
            tc.tile_pool(name="psum", bufs=2, space="PSUM") as psum,
            tc.tile_pool(name="dram", bufs=1, space="DRAM") as dram,
        ):
            nc.gpsimd.load_library(library_config.mlp)
            # constants
            ident = cpool.tile([P, P], BF16)
            make_identity(nc, ident[:])
            iota_t = cpool.tile([P, P], BF16)
            nc.sync.dma_start(out=iota_t[:], in_=iota_in[:])
            wself_t = cpool.tile([D, D], BF16)
            nc.sync.dma_start(out=wself_t[:], in_=wself[:])
            win_t = cpool.tile([D, D], BF16)
            nc.sync.dma_start(out=win_t[:], in_=win[:])
            wout_t = cpool.tile([D, D], BF16)
            nc.sync.dma_start(out=wout_t[:], in_=wout[:])
            bbc_t = cpool.tile([P, D], BF16)
            nc.sync.dma_start(out=bbc_t[:], in_=bbc[:])
            invc_d_t = cpool.tile([P, NB], F32)
            nc.sync.dma_start(out=invc_d_t[:], in_=invc_d[:])
            invc_s_t = cpool.tile([P, NB], F32)
            nc.sync.dma_start(out=invc_s_t[:], in_=invc_s[:])

            # dram buffers
            cc_in = dram.tile([NBP, D], BF16)
            cc_out = dram.tile([TROWS, D], BF16, addr_space="Shared")
            tout_loc = dram.tile([NBP, D], BF16)

            # preload all gather index tiles (tiny; keeps phase C gathers
            # from queueing behind the big stream DMAs)
            sidx_tiles = []
            for ci in range(NCH):
                n = chunks[ci][0]
                sidx = cpool.tile([P, CH // 16], I16, name=f"sidx{ci}")
                nc.sync.dma_start(out=sidx[:, : n // 16], in_=gidx[ci, :, : n // 16])
                sidx_tiles.append(sidx)

            # ---- Phase A (shared for dst and src passes) ----
            def phase_a(agat, va, KAs, invc_t, w_t, out_dram, add_bias):
                j0 = 0
                for b in range(NB):
                    KA = KAs[b]
                    if KA == 0:
                        continue
                    valt = spool.tile([P, KMAX], F32, tag="aval")
                    nc.sync.dma_start(out=valt[:, :KA], in_=va[:, j0 : j0 + KA])
                    gat = pool.tile([P, KMAX * D], BF16, tag="agather")
                    nc.sync.dma_start(
                        out=gat[:, : KA * D], in_=agat[:, j0 * D : (j0 + KA) * D]
                    )
                    ps = psum.tile([P, D], F32, tag="pA")
                    for j in range(KA):
                        oh = spool.tile([P, P], BF16, tag="oh")
                        eng = nc.vector if j % 2 == 0 else nc.gpsimd
                        eng.tensor_scalar(
                            oh[:], iota_t[:], valt[:, j : j + 1], None,
                            mybir.AluOpType.is_equal,
                        )
                        nc.tensor.matmul(
                            ps[:], oh[:], gat[:, j * D : (j + 1) * D],
                            start=(j == 0), stop=(j == KA - 1),
                        )
                    means = spool.tile([P, D], BF16, tag="means")
                    nc.scalar.mul(out=means[:], in_=ps[:], mul=invc_t[:, b : b + 1])
                    pst = psum.tile([P, D], BF16, tag="pB")
                    nc.tensor.transpose(pst[:], means[:], ident[:])
                    meansT = spool.tile([P, D], BF16, tag="meansT")
                    nc.scalar.copy(out=meansT[:], in_=pst[:])
                    psT = psum.tile([P, D], F32, tag="pC")
                    nc.tensor.matmul(psT[:], meansT[:], w_t[:], start=True, stop=True)
                    tt = spool.tile([P, D], BF16, tag="tt")
                    if add_bias:
                        nc.vector.tensor_add(tt[:], psT[:], bbc_t[:])
                    else:
                        nc.scalar.copy(out=tt[:], in_=psT[:])
                    nc.sync.dma_start(out=out_dram[b * P : (b + 1) * P, :], in_=tt[:])
                    j0 += KA

            KAd = [KL[b] + KH[b] for b in range(NB)]
            phase_a(agat_d, va_d, KAd, invc_d_t, win_t, cc_in, True)
            nc.gpsimd.collective_compute(
                "AllGather", mybir.AluOpType.bypass,
                replica_groups=[list(range(C))],
                ins=[cc_in.opt()], outs=[cc_out.opt()],
            )
            phase_a(agat_s, va_s, KS, invc_s_t, wout_t, tout_loc, False)

            if BARRIER:
                tc.strict_bb_all_engine_barrier()

            # ---- Phase C ----
            # segment list: (block b, ntiles, xt-slice tile offset) in slot order
            segs = [("lo", b, KL[b]) for b in range(NB) if KL[b] > 0] + [
                ("hi", b, KH[b]) for b in range(NB) if KH[b] > 0
            ]

            NGRP = CH // P
            gi_tiles = [None] * NCH  # chunk idx -> (tile, ngrp)
            next_chunk = 0
            chunk_slot0 = []
            acc = 0
            for n, _hi in chunks:
                chunk_slot0.append(acc)
                acc += n

            def issue_chunk(ci):
                n, hi = chunks[ci]
                ngrp = n // P
                sidx = sidx_tiles[ci]
                gi = gpool.tile([P, NGRP * D], BF16, tag="gi")
                base = HIBASE if hi else 0
                nc.gpsimd.dma_gather(
                    out_ap=gi[:, : ngrp * D].rearrange("p (g d) -> p g d", g=ngrp),
                    in_ap=cc_out[base : base + LOCAP, :],
                    idxs_ap=sidx[:, : n // 16],
                    num_idxs=n, num_idxs_reg=n, elem_size=D,
                    single_packet=False,
                )
                gi_tiles[ci] = gi

            slot = 0  # global slot cursor (tiles processed * P)
            t_glob = 0  # global tile cursor
            for kind, b, ntiles in segs:
                # make sure gather chunks covering this segment are issued
                while next_chunk < NCH and chunk_slot0[next_chunk] < slot + ntiles * P:
                    issue_chunk(next_chunk)
                    next_chunk += 1
                xT = pool.tile([P, KMAX * D], BF16, tag="xT")
                nc.sync.dma_start(
                    out=xT[:, : ntiles * D],
                    in_=agat_dt[:, t_glob * D : (t_glob + ntiles) * D],
                )
                ohT = pool.tile([P, KMAX * P], BF16, tag="ohT")
                nc.sync.dma_start(
                    out=ohT[:, : ntiles * P],
                    in_=ohts[:, t_glob * P : (t_glob + ntiles) * P],
                )
                tout_b = spool.tile([P, D], BF16, tag="toutb")
                nc.sync.dma_start(out=tout_b[:], in_=tout_loc[b * P : (b + 1) * P, :])
                yo = pool.tile([P, KMAX * D], BF16, tag="yo")
                for j in range(ntiles):
                    s = slot + j * P
                    ci = 0
                    while chunk_slot0[ci] + chunks[ci][0] <= s:
                        ci += 1
                    g = (s - chunk_slot0[ci]) // P
                    gi = gi_tiles[ci]
                    psc = psum.tile([P, D], F32, tag="pD")
                    nc.tensor.matmul(
                        psc[:], xT[:, j * D : (j + 1) * D], wself_t[:],
                        start=True, stop=False,
                    )
                    nc.tensor.matmul(
                        psc[:], ohT[:, j * P : (j + 1) * P], tout_b[:],
                        start=False, stop=True,
                    )
                    s2 = spool.tile([P, D], BF16, tag="s2")
                    nc.vector.tensor_add(s2[:], psc[:], gi[:, g * D : (g + 1) * D])
                    t1 = spool.tile([P, D], BF16, tag="t1")
                    nc.scalar.mul(out=t1[:], in_=s2[:], mul=0.2)
                    nc.vector.tensor_max(
                        yo[:, j * D : (j + 1) * D], s2[:], t1[:]
                    )
                nc.sync.dma_start(
                    out=y[:, t_glob * D : (t_glob + ntiles) * D],
                    in_=yo[:, : ntiles * D],
                )
                slot += ntiles * P
                t_glob += ntiles

    nc.compile()
    return nc


def prepare_inputs(cfg, edge_attr, edge_index, W_self_w, W_self_b, W_in_w, W_out_w):
    """Host-side sharding / graph partitioning. Returns (params, in_maps, post)."""
    E, N = cfg["E"], cfg["N"]
    NPC, NB, NBP = _derive(cfg)

    edge_attr = np.asarray(edge_attr, dtype=np.float32)
    src = np.asarray(edge_index[0], dtype=np.int64)
    dst = np.asarray(edge_index[1], dtype=np.int64)

    wself = np.ascontiguousarray(np.asarray(W_self_w, np.float32).T)
    win = np.ascontiguousarray(np.asarray(W_in_w, np.float32).T)
    wout = np.ascontiguousarray(np.asarray(W_out_w, np.float32).T)
    bbc = np.tile(np.asarray(W_self_b, dtype=np.float32)[None, :], (P, 1))
    iota = np.tile(np.arange(P, dtype=np.float32)[None, :], (P, 1))

    src_row = (src // NPC) * NBP + (src % NPC)

    # ---------- phase A-src grouping (per src owner core / block) ----------
    core_s = src // NPC
    loc_s = src - core_s * NPC
    blk_s = loc_s >> 7
    sloc = (loc_s & 127).astype(np.float32)
    key_s = core_s * NB + blk_s
    cnt_s = np.bincount(key_s, minlength=C * NB).reshape(C, NB)
    KS = [int(np.ceil(cnt_s[:, b].max() / P)) for b in range(NB)]
    TOTJS = sum(KS)

    order_s = np.argsort(key_s, kind="stable")
    starts_s = np.zeros(C * NB, dtype=np.int64)
    np.cumsum(cnt_s.ravel()[:-1], out=starts_s[1:])
    j0s = np.zeros(NB, dtype=np.int64)
    np.cumsum(np.asarray(KS[:-1]), out=j0s[1:])
    # slot within core = (j0s[blk] + pos within block) with slot=(j*128+p)
    pos_s = np.arange(E, dtype=np.int64) - starts_s[key_s[order_s]]
    slot_s = j0s[blk_s[order_s]] * P + pos_s  # slot within its core

    # ---------- phase A-dst / phase C grouping ----------
    core_d = dst // NPC
    loc_d = dst - core_d * NPC
    blk_d = loc_d >> 7
    dloc = (loc_d & 127).astype(np.int64)
    key_d = core_d * NB + blk_d
    cnt_d = np.bincount(key_d, minlength=C * NB).reshape(C, NB)
    is_lo_must = src_row < HIBASE
    is_lo_ok = src_row < LOCAP
    cnt_lo_must = np.bincount(key_d, weights=is_lo_must, minlength=C * NB
                              ).reshape(C, NB).astype(np.int64)
    cnt_lo_ok = np.bincount(key_d, weights=is_lo_ok, minlength=C * NB
                            ).reshape(C, NB).astype(np.int64)

    KL, KH = [], []
    for b in range(NB):
        kl_min = int(np.ceil(cnt_lo_must[:, b].max() / P))
        kl_max = int(np.floor(cnt_lo_ok[:, b].min() / P))
        best = None
        for kl in range(kl_min, max(kl_min, kl_max) + 1):
            nhi = np.maximum(cnt_d[:, b] - np.minimum(cnt_lo_ok[:, b], kl * P), 0)
            kh = int(np.ceil(nhi.max() / P))
            if best is None or kl + kh < best[0] + best[1]:
                best = (kl, kh)
        KL.append(best[0])
        KH.append(best[1])
    TOTJ = sum(KL) + sum(KH)
    NLO = sum(KL) * P

    # per-(core, block) edge lists
    order_d = np.argsort(key_d, kind="stable")
    starts_d = np.zeros(C * NB + 1, dtype=np.int64)
    np.cumsum(cnt_d.ravel(), out=starts_d[1:])

    # tile offsets: lo segments (block order), then hi segments
    lo_t0 = np.zeros(NB, dtype=np.int64)
    np.cumsum(np.asarray(KL[:-1]), out=lo_t0[1:])
    hi_t0 = np.zeros(NB, dtype=np.int64)
    np.cumsum(np.asarray(KH[:-1]), out=hi_t0[1:])
    hi_t0 += sum(KL)

    # slot_edge per core: global edge id at each slot, -1 = pad
    TOT_SLOTS = TOTJ * P
    slot_edge = np.full((C, TOT_SLOTS), -1, dtype=np.int64)
    for c in range(C):
        for b in range(NB):
            k = c * NB + b
            e_ids = order_d[starts_d[k] : starts_d[k + 1]]
            if len(e_ids) == 0:
                continue
            lo_mask = is_lo_ok[e_ids]
            lo_ids = e_ids[lo_mask]
            hi_ids = e_ids[~lo_mask]
            cap = KL[b] * P
            if len(lo_ids) > cap:
                # move overlap edges (src_row >= HIBASE) to hi until it fits
                movable = src_row[lo_ids] >= HIBASE
                mv_idx = np.where(movable)[0]
                nmove = len(lo_ids) - cap
                mv = mv_idx[:nmove]
                keep = np.ones(len(lo_ids), dtype=bool)
                keep[mv] = False
                hi_ids = np.concatenate([hi_ids, lo_ids[~keep]])
                lo_ids = lo_ids[keep]
            assert len(hi_ids) <= KH[b] * P, (b, len(hi_ids), KH[b])
            s0 = lo_t0[b] * P
            slot_edge[c, s0 : s0 + len(lo_ids)] = lo_ids
            s0 = hi_t0[b] * P
            slot_edge[c, s0 : s0 + len(hi_ids)] = hi_ids

    # gather chunks
    chunks = []
    off = 0
    NHI = sum(KH) * P
    while off < NLO:
        chunks.append(min(CH, NLO - off))
        off += CH
    off = 0
    while off < NHI:
        chunks.append(min(CH, NHI - off))
        off += CH
    NCH = len(chunks)

    def wrap_idx(vals):
        n = len(vals)
        t = np.zeros((16, CH // 16), dtype=np.int16)
        t[np.arange(n) % 16, np.arange(n) // 16] = vals.astype(np.int16)
        return np.tile(t, (8, 1))

    # phase A-dst uses its own block-contiguous tile order (lo+hi per block);
    # phase C arrays stay in slot order ([all lo][all hi]).
    tile_perm = np.concatenate(
        [
            np.concatenate([
                np.arange(lo_t0[b], lo_t0[b] + KL[b]),
                np.arange(hi_t0[b], hi_t0[b] + KH[b]),
            ])
            for b in range(NB)
        ]
    ).astype(np.int64)

    in_maps = []
    for c in range(C):
        se = slot_edge[c]
        valid = se >= 0
        ge = np.where(valid, se, 0)

        xs = np.where(valid[:, None], edge_attr[ge], 0).astype(np.float32)
        xs3 = xs.reshape(TOTJ, P, D)
        agat_c = np.ascontiguousarray(
            xs3[tile_perm].transpose(1, 0, 2).reshape(P, TOTJ * D)).astype(BF16NP)
        agat_ct = np.ascontiguousarray(
            xs3.transpose(2, 0, 1).reshape(P, TOTJ * P)).astype(BF16NP)

        dv = np.where(valid, dloc[ge], -1)
        oht = np.zeros((TOTJ, P, P), dtype=np.float32)
        sl = np.arange(TOT_SLOTS)[valid]
        oht[sl // P, dv[valid], sl % P] = 1.0
        oht_c = np.ascontiguousarray(
            oht.transpose(1, 0, 2).reshape(P, TOTJ * P)).astype(BF16NP)
        va_c = np.ascontiguousarray(
            dv.reshape(TOTJ, P)[tile_perm].T.astype(np.float32))

        # gather indices
        gidx_full = np.where(valid, src_row[ge], 0)
        gidx_full[NLO:] = np.where(valid[NLO:], gidx_full[NLO:] - HIBASE, 0)
        assert gidx_full.min() >= 0 and gidx_full.max() < LOCAP
        gx = np.zeros((NCH, P, CH // 16), dtype=np.int16)
        off = 0
        for ci, n in enumerate(chunks):
            gx[ci] = wrap_idx(gidx_full[off : off + n])
            off += n

        # phase A-src arrays
        m = core_s[order_s] == c
        sslot = slot_s[m]
        sids = order_s[m]
        xs_s = np.zeros((TOTJS * P, D), dtype=np.float32)
        xs_s[sslot] = edge_attr[sids]
        va_sf = np.full(TOTJS * P, -1.0, dtype=np.float32)
        va_sf[sslot] = sloc[sids]
        agat_s_c = np.ascontiguousarray(
            xs_s.reshape(TOTJS, P, D).transpose(1, 0, 2).reshape(P, TOTJS * D)
        ).astype(BF16NP)
        va_s_c = np.ascontiguousarray(va_sf.reshape(TOTJS, P).T)

        # inverse counts
        def build_inv(node_of_edge):
            cnt = np.bincount(node_of_edge, minlength=N).astype(np.float32)
            inv = 1.0 / np.maximum(cnt, 1.0)
            pad = np.zeros(NBP, dtype=np.float32)
            pad[:NPC] = inv[c * NPC : (c + 1) * NPC]
            return np.ascontiguousarray(pad.reshape(NB, P).T)

        in_maps.append(
            dict(
                agat_d=agat_c, agat_dt=agat_ct, ohts=oht_c, va_d=va_c,
                agat_s=agat_s_c, va_s=va_s_c,
                invc_d=build_inv(dst), invc_s=build_inv(src),
                gidx=gx,
                wself=wself.astype(BF16NP), win=win.astype(BF16NP),
                wout=wout.astype(BF16NP), bbc=bbc.astype(BF16NP),
                iota=iota.astype(BF16NP),
            )
        )

    slot_edge_all = slot_edge

    def postprocess(results):
        full = np.empty((E, D), dtype=np.float32)
        for c in range(C):
            yv = results[c]["y"].astype(np.float32)
            yv = yv.reshape(P, TOTJ, D).transpose(1, 0, 2).reshape(TOT_SLOTS, D)
            se = slot_edge_all[c]
            valid = se >= 0
            full[se[valid]] = yv[valid]
        return full

    params = (tuple(KL), tuple(KH), tuple(KS))
    return params, in_maps, postprocess


_NC_CACHE = {}


def run(cfg, inputs, trace=False, trace_kwargs=None):
    params, in_maps, post = prepare_inputs(
        cfg,
        inputs["edge_attr"],
        inputs["edge_index"],
        inputs["W_self_w"],
        inputs["W_self_b"],
        inputs["W_in_w"],
        inputs["W_out_w"],
    )
    key = (tuple(sorted(cfg.items())), params)
    if key not in _NC_CACHE:
        _NC_CACHE[key] = build_kernel(cfg, list(params[0]), list(params[1]),
                                      list(params[2]))
    nc = _NC_CACHE[key]
    kw = {}
    if trace:
        kw["trace"] = True
        if trace_kwargs:
            kw.update(trace_kwargs)
    res = run_bass_kernel_spmd(nc, in_maps, core_ids=list(range(C)), **kw)
    return post(res.results), res


def kernel(**inputs) -> np.ndarray:
    out, _ = run(_cfg_full(), inputs)
    return out.astype(np.float32)


# revision 11
# speedup vs baseline: 2.0519x; 2.0519x over previous
"""DirectedEdgeConv (gnn_message_passing) Trainium2 kernel, 8-core SPMD.

out[e] = leaky_relu(edge_attr[e] @ Wself^T + b
                    + T_in[src[e]] + T_out[dst[e]], 0.2)
where T_in  = scatter_mean(edge_attr, dst) @ Win^T + b   [node table]
      T_out = scatter_mean(edge_attr, src) @ Wout^T      [node table]

Sharding / algorithm (v2):
  Core c owns nodes [c*NPC, (c+1)*NPC).  Edges are assigned to cores twice:
  by dst owner (phase A-dst + phase C) and by src owner (phase A-src).

  Phase A-dst (dst-block-grouped edge stream): segment-sum via one-hot
  matmuls -> T_in slice for own nodes (bias folded in) -> ONE AllGather
  (bf16) -> full T_in table on every core.
  Phase A-src: same grouping by src -> T_out slice for own nodes; stays
  LOCAL (phase C only ever needs the core's own T_out rows!).
  Phase C (same dst-block-grouped stream): per 128-edge tile
    psum  = xT_tile.T @ Wself'          (h_self)
    psum += ohT_tile.T @ Tout_block     (T_out[dst] via host-built one-hot)
    y     = Lrelu(psum + gi)            (gi = dma_gather of T_in[src])
  The only per-edge random access left is the T_in gather (bf16 256B rows,
  int16 indices; edges are pre-split lo/hi against two overlapping 32768-row
  table windows so indices fit 15 bits).  Gathers are issued in 4096-index
  chunks to amortize the GpSimd SWDGE emission cost.
"""

import os
import sys

sys.path.insert(0, "/opt/trn_rl_repo")

import numpy as np
import ml_dtypes

BF16NP = ml_dtypes.bfloat16

import concourse.bacc as bacc
import concourse.bass as bass
import concourse.mybir as mybir
import concourse.tile as tile
from concourse import library_config
from concourse.bass_utils import run_bass_kernel_spmd
from concourse.masks import make_identity

P = 128
D = 128
C = 8
HIBASE = 17408   # hi table window starts here; both windows are 32768 rows
LOCAP = 32768
CH = 4096        # gather chunk size (indices per dma_gather call)

F32 = mybir.dt.float32
BF16 = mybir.dt.bfloat16
I16 = mybir.dt.int16

BARRIER = os.environ.get("KBARRIER", "1") == "1"


def _cfg_full():
    return dict(E=600000, N=50000)


def _derive(cfg):
    E, N = cfg["E"], cfg["N"]
    assert N % C == 0
    NPC = N // C
    NB = (NPC + P - 1) // P
    NBP = NB * P
    return NPC, NB, NBP


def build_kernel(cfg, KL, KH, KS):
    """KL/KH: per-dst-block lo/hi tile counts (len NB). KS: per-src-block
    tile counts for phase A-src (len NB). All uniform across cores."""
    E, N = cfg["E"], cfg["N"]
    NPC, NB, NBP = _derive(cfg)
    TROWS = C * NBP

    TOTJ = sum(KL) + sum(KH)      # phase C / A-dst tiles per core
    TOTJS = sum(KS)               # phase A-src tiles per core
    NLO = sum(KL) * P
    NHI = sum(KH) * P
    KMAX = max(max(KL) + max(KH), max(KS))

    # gather chunks: (num_idxs, hi?) list; slot offsets implicit/sequential
    chunks = []
    off = 0
    while off < NLO:
        n = min(CH, NLO - off)
        chunks.append((n, 0))
        off += n
    off = 0
    while off < NHI:
        n = min(CH, NHI - off)
        chunks.append((n, 1))
        off += n
    NCH = len(chunks)

    nc = bacc.Bacc(None, target_bir_lowering=False, debug=False,
                   num_swdge_queues=4)

    # ---- I/O ----
    agat_d = nc.dram_tensor("agat_d", [P, TOTJ * D], BF16, kind="ExternalInput")
    agat_dt = nc.dram_tensor("agat_dt", [P, TOTJ * D], BF16, kind="ExternalInput")
    ohts = nc.dram_tensor("ohts", [P, TOTJ * P], BF16, kind="ExternalInput")
    va_d = nc.dram_tensor("va_d", [P, TOTJ], F32, kind="ExternalInput")
    agat_s = nc.dram_tensor("agat_s", [P, TOTJS * D], BF16, kind="ExternalInput")
    va_s = nc.dram_tensor("va_s", [P, TOTJS], F32, kind="ExternalInput")
    invc_d = nc.dram_tensor("invc_d", [P, NB], F32, kind="ExternalInput")
    invc_s = nc.dram_tensor("invc_s", [P, NB], F32, kind="ExternalInput")
    gidx = nc.dram_tensor("gidx", [NCH, P, CH // 16], I16, kind="ExternalInput")
    wself = nc.dram_tensor("wself", [D, D], BF16, kind="ExternalInput")
    win = nc.dram_tensor("win", [D, D], BF16, kind="ExternalInput")
    wout = nc.dram_tensor("wout", [D, D], BF16, kind="ExternalInput")
    bbc = nc.dram_tensor("bbc", [P, D], BF16, kind="ExternalInput")
    iota_in = nc.dram_tensor("iota", [P, P], F32, kind="ExternalInput")
    y = nc.dram_tensor("y", [P, TOTJ * D], BF16, kind="ExternalOutput")

    with tile.TileContext(nc) as tc:
        with (
            tc.tile_pool(name="const", bufs=1) as cpool,
            tc.tile_pool(name="sbuf", bufs=3) as pool,
            tc.tile_pool(name="gpool", bufs=8) as gpool,
            tc.tile_pool(name="small", bufs=4) as spool,
            tc.tile_pool(name="psum", bufs=2, space="PSUM") as psum,
            tc.tile_pool(name="dram", bufs=1, space="DRAM") as dram,
        ):
            nc.gpsimd.load_library(library_config.mlp)
            # constants
            ident = cpool.tile([P, P], BF16)
            make_identity(nc, ident[:])
            iota_t = cpool.tile([P, P], F32)
            nc.sync.dma_start(out=iota_t[:], in_=iota_in[:])
            wself_t = cpool.tile([D, D], BF16)
            nc.sync.dma_start(out=wself_t[:], in_=wself[:])
            win_t = cpool.tile([D, D], BF16)
            nc.sync.dma_start(out=win_t[:], in_=win[:])
            wout_t = cpool.tile([D, D], BF16)
            nc.sync.dma_start(out=wout_t[:], in_=wout[:])
            bbc_t = cpool.tile([P, D], BF16)
            nc.sync.dma_start(out=bbc_t[:], in_=bbc[:])
            invc_d_t = cpool.tile([P, NB], F32)
            nc.sync.dma_start(out=invc_d_t[:], in_=invc_d[:])
            invc_s_t = cpool.tile([P, NB], F32)
            nc.sync.dma_start(out=invc_s_t[:], in_=invc_s[:])

            # dram buffers
            cc_in = dram.tile([NBP, D], BF16)
            cc_out = dram.tile([TROWS, D], BF16, addr_space="Shared")
            tout_loc = dram.tile([NBP, D], BF16)

            # preload all gather index tiles (tiny; keeps phase C gathers
            # from queueing behind the big stream DMAs)
            sidx_tiles = []
            for ci in range(NCH):
                n = chunks[ci][0]
                sidx = cpool.tile([P, CH // 16], I16, name=f"sidx{ci}")
                nc.sync.dma_start(out=sidx[:, : n // 16], in_=gidx[ci, :, : n // 16])
                sidx_tiles.append(sidx)

            NGRP = CH // P
            NQ = 4
            PREAG = 8  # chunks whose descriptors are emitted during phase A
            gi_tiles = [None] * NCH
            chunk_slot0 = []
            acc = 0
            for n, _hi in chunks:
                chunk_slot0.append(acc)
                acc += n
            gsems = [nc.alloc_semaphore(f"gsem{i}") for i in range(NCH)]

            def prep_chunk(ci):
                n, hi = chunks[ci]
                ngrp = n // P
                gi = gpool.tile([P, NGRP * D], BF16, tag="gi")
                base = HIBASE if hi else 0
                nc.gpsimd.dma_gather(
                    out_ap=gi[:, : ngrp * D].rearrange("p (g d) -> p g d", g=ngrp),
                    in_ap=cc_out[base : base + LOCAP, :],
                    idxs_ap=sidx_tiles[ci][:, : n // 16],
                    num_idxs=n, num_idxs_reg=n, elem_size=D,
                    single_packet=False,
                    prepare_only=True, sem=gsems[ci], queue_num=ci % NQ,
                )
                gi_tiles[ci] = gi

            # emit gather descriptors for the first PREAG chunks while the
            # compute engines run phase A (descriptors have no data deps;
            # the table read is deferred to the trigger)
            for ci in range(min(PREAG, NCH)):
                prep_chunk(ci)

            # ---- Phase A (shared for dst and src passes) ----
            def phase_a(agat, va, KAs, invc_t, w_t, out_dram, add_bias):
                j0 = 0
                for b in range(NB):
                    KA = KAs[b]
                    if KA == 0:
                        continue
                    valt = spool.tile([P, KMAX], F32, tag="aval")
                    nc.sync.dma_start(out=valt[:, :KA], in_=va[:, j0 : j0 + KA])
                    gat = pool.tile([P, KMAX * D], BF16, tag="agather")
                    nc.sync.dma_start(
                        out=gat[:, : KA * D], in_=agat[:, j0 * D : (j0 + KA) * D]
                    )
                    ps = psum.tile([P, D], F32, tag="pA")
                    for j in range(KA):
                        oh = spool.tile([P, P], BF16, tag="oh")
                        nc.vector.tensor_scalar(
                            oh[:], iota_t[:], valt[:, j : j + 1], None,
                            mybir.AluOpType.is_equal,
                        )
                        nc.tensor.matmul(
                            ps[:], oh[:], gat[:, j * D : (j + 1) * D],
                            start=(j == 0), stop=(j == KA - 1),
                        )
                    means = spool.tile([P, D], BF16, tag="means")
                    nc.scalar.mul(out=means[:], in_=ps[:], mul=invc_t[:, b : b + 1])
                    pst = psum.tile([P, D], BF16, tag="pB")
                    nc.tensor.transpose(pst[:], means[:], ident[:])
                    meansT = spool.tile([P, D], BF16, tag="meansT")
                    nc.scalar.copy(out=meansT[:], in_=pst[:])
                    psT = psum.tile([P, D], F32, tag="pC")
                    nc.tensor.matmul(psT[:], meansT[:], w_t[:], start=True, stop=True)
                    tt = spool.tile([P, D], BF16, tag="tt")
                    if add_bias:
                        nc.vector.tensor_add(tt[:], psT[:], bbc_t[:])
                    else:
                        nc.scalar.copy(out=tt[:], in_=psT[:])
                    nc.sync.dma_start(out=out_dram[b * P : (b + 1) * P, :], in_=tt[:])
                    j0 += KA

            KAd = [KL[b] + KH[b] for b in range(NB)]
            phase_a(agat_d, va_d, KAd, invc_d_t, win_t, cc_in, True)
            nc.gpsimd.collective_compute(
                "AllGather", mybir.AluOpType.bypass,
                replica_groups=[list(range(C))],
                ins=[cc_in.opt()], outs=[cc_out.opt()],
            )
            phase_a(agat_s, va_s, KS, invc_s_t, wout_t, tout_loc, False)

            if BARRIER:
                tc.strict_bb_all_engine_barrier()

            # fire the pre-emitted gathers (waits for the AllGather via the
            # deferred read dep on cc_out)
            for q in range(min(NQ, PREAG, NCH)):
                nc.gpsimd.trigger_dma(count=None, queue_num=q)
            next_chunk = min(PREAG, NCH)

            # ---- Phase C ----
            # segment list: (block b, ntiles, xt-slice tile offset) in slot order
            segs = [("lo", b, KL[b]) for b in range(NB) if KL[b] > 0] + [
                ("hi", b, KH[b]) for b in range(NB) if KH[b] > 0
            ]


            slot = 0  # global slot cursor (tiles processed * P)
            t_glob = 0  # global tile cursor
            for kind, b, ntiles in segs:
                # make sure gather chunks covering this segment are issued
                while (next_chunk < NCH
                       and chunk_slot0[next_chunk] < slot + ntiles * P + 2 * CH):
                    prep_chunk(next_chunk)
                    nc.gpsimd.trigger_dma(count=None, queue_num=next_chunk % NQ)
                    next_chunk += 1
                xT = pool.tile([P, KMAX * D], BF16, tag="xT")
                nc.sync.dma_start(
                    out=xT[:, : ntiles * D],
                    in_=agat_dt[:, t_glob * D : (t_glob + ntiles) * D],
                )
                ohT = pool.tile([P, KMAX * P], BF16, tag="ohT")
                nc.sync.dma_start(
                    out=ohT[:, : ntiles * P],
                    in_=ohts[:, t_glob * P : (t_glob + ntiles) * P],
                )
                tout_b = spool.tile([P, D], BF16, tag="toutb")
                nc.sync.dma_start(out=tout_b[:], in_=tout_loc[b * P : (b + 1) * P, :])
                yo = pool.tile([P, KMAX * D], BF16, tag="yo")
                for j in range(ntiles):
                    s = slot + j * P
                    ci = 0
                    while chunk_slot0[ci] + chunks[ci][0] <= s:
                        ci += 1
                    g = (s - chunk_slot0[ci]) // P
                    gi = gi_tiles[ci]
                    psc = psum.tile([P, D], F32, tag="pD")
                    nc.tensor.matmul(
                        psc[:], xT[:, j * D : (j + 1) * D], wself_t[:],
                        start=True, stop=False,
                    )
                    nc.tensor.matmul(
                        psc[:], ohT[:, j * P : (j + 1) * P], tout_b[:],
                        start=False, stop=True,
                    )
                    s2 = spool.tile([P, D], BF16, tag="s2")
                    nc.vector.tensor_add(s2[:], psc[:], gi[:, g * D : (g + 1) * D])
                    t1 = spool.tile([P, D], BF16, tag="t1")
                    nc.scalar.mul(out=t1[:], in_=s2[:], mul=0.2)
                    nc.vector.tensor_max(
                        yo[:, j * D : (j + 1) * D], s2[:], t1[:]
                    )
                nc.sync.dma_start(
                    out=y[:, t_glob * D : (t_glob + ntiles) * D],
                    in_=yo[:, : ntiles * D],
                )
                slot += ntiles * P
                t_glob += ntiles

    nc.compile()
    return nc


def prepare_inputs(cfg, edge_attr, edge_index, W_self_w, W_self_b, W_in_w, W_out_w):
    """Host-side sharding / graph partitioning. Returns (params, in_maps, post)."""
    E, N = cfg["E"], cfg["N"]
    NPC, NB, NBP = _derive(cfg)

    edge_attr = np.asarray(edge_attr, dtype=np.float32)
    src = np.asarray(edge_index[0], dtype=np.int64)
    dst = np.asarray(edge_index[1], dtype=np.int64)

    wself = np.ascontiguousarray(np.asarray(W_self_w, np.float32).T)
    win = np.ascontiguousarray(np.asarray(W_in_w, np.float32).T)
    wout = np.ascontiguousarray(np.asarray(W_out_w, np.float32).T)
    bbc = np.tile(np.asarray(W_self_b, dtype=np.float32)[None, :], (P, 1))
    iota = np.tile(np.arange(P, dtype=np.float32)[None, :], (P, 1))

    src_row = (src // NPC) * NBP + (src % NPC)

    # ---------- phase A-src grouping (per src owner core / block) ----------
    core_s = src // NPC
    loc_s = src - core_s * NPC
    blk_s = loc_s >> 7
    sloc = (loc_s & 127).astype(np.float32)
    key_s = core_s * NB + blk_s
    cnt_s = np.bincount(key_s, minlength=C * NB).reshape(C, NB)
    KS = [int(np.ceil(cnt_s[:, b].max() / P)) for b in range(NB)]
    TOTJS = sum(KS)

    order_s = np.argsort(key_s, kind="stable")
    starts_s = np.zeros(C * NB, dtype=np.int64)
    np.cumsum(cnt_s.ravel()[:-1], out=starts_s[1:])
    j0s = np.zeros(NB, dtype=np.int64)
    np.cumsum(np.asarray(KS[:-1]), out=j0s[1:])
    # slot within core = (j0s[blk] + pos within block) with slot=(j*128+p)
    pos_s = np.arange(E, dtype=np.int64) - starts_s[key_s[order_s]]
    slot_s = j0s[blk_s[order_s]] * P + pos_s  # slot within its core

    # ---------- phase A-dst / phase C grouping ----------
    core_d = dst // NPC
    loc_d = dst - core_d * NPC
    blk_d = loc_d >> 7
    dloc = (loc_d & 127).astype(np.int64)
    key_d = core_d * NB + blk_d
    cnt_d = np.bincount(key_d, minlength=C * NB).reshape(C, NB)
    is_lo_must = src_row < HIBASE
    is_lo_ok = src_row < LOCAP
    cnt_lo_must = np.bincount(key_d, weights=is_lo_must, minlength=C * NB
                              ).reshape(C, NB).astype(np.int64)
    cnt_lo_ok = np.bincount(key_d, weights=is_lo_ok, minlength=C * NB
                            ).reshape(C, NB).astype(np.int64)

    KL, KH = [], []
    for b in range(NB):
        kl_min = int(np.ceil(cnt_lo_must[:, b].max() / P))
        kl_max = int(np.floor(cnt_lo_ok[:, b].min() / P))
        best = None
        for kl in range(kl_min, max(kl_min, kl_max) + 1):
            nhi = np.maximum(cnt_d[:, b] - np.minimum(cnt_lo_ok[:, b], kl * P), 0)
            kh = int(np.ceil(nhi.max() / P))
            if best is None or kl + kh < best[0] + best[1]:
                best = (kl, kh)
        KL.append(best[0])
        KH.append(best[1])
    TOTJ = sum(KL) + sum(KH)
    NLO = sum(KL) * P

    # per-(core, block) edge lists
    order_d = np.argsort(key_d, kind="stable")
    starts_d = np.zeros(C * NB + 1, dtype=np.int64)
    np.cumsum(cnt_d.ravel(), out=starts_d[1:])

    # tile offsets: lo segments (block order), then hi segments
    lo_t0 = np.zeros(NB, dtype=np.int64)
    np.cumsum(np.asarray(KL[:-1]), out=lo_t0[1:])
    hi_t0 = np.zeros(NB, dtype=np.int64)
    np.cumsum(np.asarray(KH[:-1]), out=hi_t0[1:])
    hi_t0 += sum(KL)

    # slot_edge per core: global edge id at each slot, -1 = pad
    TOT_SLOTS = TOTJ * P
    slot_edge = np.full((C, TOT_SLOTS), -1, dtype=np.int64)
    for c in range(C):
        for b in range(NB):
            k = c * NB + b
            e_ids = order_d[starts_d[k] : starts_d[k + 1]]
            if len(e_ids) == 0:
                continue
            lo_mask = is_lo_ok[e_ids]
            lo_ids = e_ids[lo_mask]
            hi_ids = e_ids[~lo_mask]
            cap = KL[b] * P
            if len(lo_ids) > cap:
                # move overlap edges (src_row >= HIBASE) to hi until it fits
                movable = src_row[lo_ids] >= HIBASE
                mv_idx = np.where(movable)[0]
                nmove = len(lo_ids) - cap
                mv = mv_idx[:nmove]
                keep = np.ones(len(lo_ids), dtype=bool)
                keep[mv] = False
                hi_ids = np.concatenate([hi_ids, lo_ids[~keep]])
                lo_ids = lo_ids[keep]
            assert len(hi_ids) <= KH[b] * P, (b, len(hi_ids), KH[b])
            s0 = lo_t0[b] * P
            slot_edge[c, s0 : s0 + len(lo_ids)] = lo_ids
            s0 = hi_t0[b] * P
            slot_edge[c, s0 : s0 + len(hi_ids)] = hi_ids

    # gather chunks
    chunks = []
    off = 0
    NHI = sum(KH) * P
    while off < NLO:
        chunks.append(min(CH, NLO - off))
        off += CH
    off = 0
    while off < NHI:
        chunks.append(min(CH, NHI - off))
        off += CH
    NCH = len(chunks)

    def wrap_idx(vals):
        n = len(vals)
        t = np.zeros((16, CH // 16), dtype=np.int16)
        t[np.arange(n) % 16, np.arange(n) // 16] = vals.astype(np.int16)
        return np.tile(t, (8, 1))

    # phase A-dst uses its own block-contiguous tile order (lo+hi per block);
    # phase C arrays stay in slot order ([all lo][all hi]).
    tile_perm = np.concatenate(
        [
            np.concatenate([
                np.arange(lo_t0[b], lo_t0[b] + KL[b]),
                np.arange(hi_t0[b], hi_t0[b] + KH[b]),
            ])
            for b in range(NB)
        ]
    ).astype(np.int64)

    in_maps = []
    for c in range(C):
        se = slot_edge[c]
        valid = se >= 0
        ge = np.where(valid, se, 0)

        xs = np.where(valid[:, None], edge_attr[ge], 0).astype(np.float32)
        xs3 = xs.reshape(TOTJ, P, D)
        agat_c = np.ascontiguousarray(
            xs3[tile_perm].transpose(1, 0, 2).reshape(P, TOTJ * D)).astype(BF16NP)
        agat_ct = np.ascontiguousarray(
            xs3.transpose(2, 0, 1).reshape(P, TOTJ * P)).astype(BF16NP)

        dv = np.where(valid, dloc[ge], -1)
        oht = np.zeros((TOTJ, P, P), dtype=np.float32)
        sl = np.arange(TOT_SLOTS)[valid]
        oht[sl // P, dv[valid], sl % P] = 1.0
        oht_c = np.ascontiguousarray(
            oht.transpose(1, 0, 2).reshape(P, TOTJ * P)).astype(BF16NP)
        va_c = np.ascontiguousarray(
            dv.reshape(TOTJ, P)[tile_perm].T.astype(np.float32))

        # gather indices
        gidx_full = np.where(valid, src_row[ge], 0)
        gidx_full[NLO:] = np.where(valid[NLO:], gidx_full[NLO:] - HIBASE, 0)
        assert gidx_full.min() >= 0 and gidx_full.max() < LOCAP
        gx = np.zeros((NCH, P, CH // 16), dtype=np.int16)
        off = 0
        for ci, n in enumerate(chunks):
            gx[ci] = wrap_idx(gidx_full[off : off + n])
            off += n

        # phase A-src arrays
        m = core_s[order_s] == c
        sslot = slot_s[m]
        sids = order_s[m]
        xs_s = np.zeros((TOTJS * P, D), dtype=np.float32)
        xs_s[sslot] = edge_attr[sids]
        va_sf = np.full(TOTJS * P, -1.0, dtype=np.float32)
        va_sf[sslot] = sloc[sids]
        agat_s_c = np.ascontiguousarray(
            xs_s.reshape(TOTJS, P, D).transpose(1, 0, 2).reshape(P, TOTJS * D)
        ).astype(BF16NP)
        va_s_c = np.ascontiguousarray(va_sf.reshape(TOTJS, P).T)

        # inverse counts
        def build_inv(node_of_edge):
            cnt = np.bincount(node_of_edge, minlength=N).astype(np.float32)
            inv = 1.0 / np.maximum(cnt, 1.0)
            pad = np.zeros(NBP, dtype=np.float32)
            pad[:NPC] = inv[c * NPC : (c + 1) * NPC]
            return np.ascontiguousarray(pad.reshape(NB, P).T)

        in_maps.append(
            dict(
                agat_d=agat_c, agat_dt=agat_ct, ohts=oht_c, va_d=va_c,
                agat_s=agat_s_c, va_s=va_s_c,
                invc_d=build_inv(dst), invc_s=build_inv(src),
                gidx=gx,
                wself=wself.astype(BF16NP), win=win.astype(BF16NP),
                wout=wout.astype(BF16NP), bbc=bbc.astype(BF16NP),
                iota=iota,
            )
        )

    slot_edge_all = slot_edge

    def postprocess(results):
        full = np.empty((E, D), dtype=np.float32)
        for c in range(C):
            yv = results[c]["y"].astype(np.float32)
            yv = yv.reshape(P, TOTJ, D).transpose(1, 0, 2).reshape(TOT_SLOTS, D)
            se = slot_edge_all[c]
            valid = se >= 0
            full[se[valid]] = yv[valid]
        return full

    params = (tuple(KL), tuple(KH), tuple(KS))
    return params, in_maps, postprocess


_NC_CACHE = {}


def run(cfg, inputs, trace=False, trace_kwargs=None):
    params, in_maps, post = prepare_inputs(
        cfg,
        inputs["edge_attr"],
        inputs["edge_index"],
        inputs["W_self_w"],
        inputs["W_self_b"],
        inputs["W_in_w"],
        inputs["W_out_w"],
    )
    key = (tuple(sorted(cfg.items())), params)
    if key not in _NC_CACHE:
        _NC_CACHE[key] = build_kernel(cfg, list(params[0]), list(params[1]),
                                      list(params[2]))
    nc = _NC_CACHE[key]
    kw = {}
    if trace:
        kw["trace"] = True
        if trace_kwargs:
            kw.update(trace_kwargs)
    res = run_bass_kernel_spmd(nc, in_maps, core_ids=list(range(C)), **kw)
    return post(res.results), res


def kernel(**inputs) -> np.ndarray:
    out, _ = run(_cfg_full(), inputs)
    return out.astype(np.float32)


# revision 19
# speedup vs baseline: 2.1088x; 1.0277x over previous
"""DirectedEdgeConv (gnn_message_passing) Trainium2 kernel, 8-core SPMD.

out[e] = leaky_relu(edge_attr[e] @ Wself^T + b
                    + T_in[src[e]] + T_out[dst[e]], 0.2)
where T_in  = scatter_mean(edge_attr, dst) @ Win^T + b   [node table]
      T_out = scatter_mean(edge_attr, src) @ Wout^T      [node table]

Sharding / algorithm (v2):
  Core c owns nodes [c*NPC, (c+1)*NPC).  Edges are assigned to cores twice:
  by dst owner (phase A-dst + phase C) and by src owner (phase A-src).

  Phase A-dst (dst-block-grouped edge stream): segment-sum via one-hot
  matmuls -> T_in slice for own nodes (bias folded in) -> ONE AllGather
  (bf16) -> full T_in table on every core.
  Phase A-src: same grouping by src -> T_out slice for own nodes; stays
  LOCAL (phase C only ever needs the core's own T_out rows!).
  Phase C (same dst-block-grouped stream): per 128-edge tile
    psum  = xT_tile.T @ Wself'          (h_self)
    psum += ohT_tile.T @ Tout_block     (T_out[dst] via host-built one-hot)
    y     = Lrelu(psum + gi)            (gi = dma_gather of T_in[src])
  The only per-edge random access left is the T_in gather (bf16 256B rows,
  int16 indices; edges are pre-split lo/hi against two overlapping 32768-row
  table windows so indices fit 15 bits).  Gathers are issued in 4096-index
  chunks to amortize the GpSimd SWDGE emission cost.
"""

import os
import sys

sys.path.insert(0, "/opt/trn_rl_repo")

import numpy as np
import ml_dtypes

BF16NP = ml_dtypes.bfloat16

import concourse.bacc as bacc
import concourse.bass as bass
import concourse.mybir as mybir
import concourse.tile as tile
from concourse import library_config
from concourse.bass_utils import run_bass_kernel_spmd
from concourse.masks import make_identity

P = 128
D = 128
C = 8
HIBASE = 17408   # hi table window starts here; both windows are 32768 rows
LOCAP = 32768
CH = 4096        # gather chunk size (indices per dma_gather call)

F32 = mybir.dt.float32
BF16 = mybir.dt.bfloat16
I16 = mybir.dt.int16

BARRIER = os.environ.get("KBARRIER", "0") == "1"


def _cfg_full():
    return dict(E=600000, N=50000)


def _derive(cfg):
    E, N = cfg["E"], cfg["N"]
    assert N % C == 0
    NPC = N // C
    NB = (NPC + P - 1) // P
    NBP = NB * P
    return NPC, NB, NBP


def build_kernel(cfg, KL, KH, KS):
    """KL/KH: per-dst-block lo/hi tile counts (len NB). KS: per-src-block
    tile counts for phase A-src (len NB). All uniform across cores."""
    E, N = cfg["E"], cfg["N"]
    NPC, NB, NBP = _derive(cfg)
    TROWS = C * NBP

    TOTJ = sum(KL) + sum(KH)      # phase C / A-dst tiles per core
    TOTJS = sum(KS)               # phase A-src tiles per core
    NLO = sum(KL) * P
    NHI = sum(KH) * P
    KMAX = max(max(KL) + max(KH), max(KS))

    # gather chunks: (num_idxs, hi?) list; slot offsets implicit/sequential
    chunks = []
    off = 0
    while off < NLO:
        n = min(CH, NLO - off)
        chunks.append((n, 0))
        off += n
    off = 0
    while off < NHI:
        n = min(CH, NHI - off)
        chunks.append((n, 1))
        off += n
    NCH = len(chunks)

    nc = bacc.Bacc(None, target_bir_lowering=False, debug=False,
                   num_swdge_queues=4, dynamic_dma_scratch_size=65536)

    # ---- I/O ----
    agat_d = nc.dram_tensor("agat_d", [P, TOTJ * D], BF16, kind="ExternalInput")
    agat_dt = nc.dram_tensor("agat_dt", [P, TOTJ * D], BF16, kind="ExternalInput")
    ohts = nc.dram_tensor("ohts", [P, TOTJ * P], BF16, kind="ExternalInput")
    ohd = nc.dram_tensor("ohd", [P, TOTJ * P], BF16, kind="ExternalInput")
    agat_s = nc.dram_tensor("agat_s", [P, TOTJS * D], BF16, kind="ExternalInput")
    va_s = nc.dram_tensor("va_s", [P, TOTJS], F32, kind="ExternalInput")
    invc_d = nc.dram_tensor("invc_d", [P, NB], F32, kind="ExternalInput")
    invc_s = nc.dram_tensor("invc_s", [P, NB], F32, kind="ExternalInput")
    gidx = nc.dram_tensor("gidx", [P, NCH * (CH // 16)], I16, kind="ExternalInput")
    wself = nc.dram_tensor("wself", [D, D], BF16, kind="ExternalInput")
    win = nc.dram_tensor("win", [D, D], BF16, kind="ExternalInput")
    wout = nc.dram_tensor("wout", [D, D], BF16, kind="ExternalInput")
    bbc = nc.dram_tensor("bbc", [P, D], BF16, kind="ExternalInput")
    iota_in = nc.dram_tensor("iota", [P, P], F32, kind="ExternalInput")
    y = nc.dram_tensor("y", [P, TOTJ * D], BF16, kind="ExternalOutput")

    with tile.TileContext(nc) as tc:
        with (
            tc.tile_pool(name="const", bufs=1) as cpool,
            tc.tile_pool(name="sbuf", bufs=3) as pool,
            tc.tile_pool(name="gpool", bufs=6) as gpool,
            tc.tile_pool(name="small", bufs=4) as spool,
            tc.tile_pool(name="psum", bufs=2, space="PSUM") as psum,
            tc.tile_pool(name="dram", bufs=1, space="DRAM") as dram,
        ):
            nc.gpsimd.load_library(library_config.mlp)
            # constants
            ident = cpool.tile([P, P], BF16)
            make_identity(nc, ident[:])
            iota_t = cpool.tile([P, P], F32)
            nc.sync.dma_start(out=iota_t[:], in_=iota_in[:])
            wself_t = cpool.tile([D, D], BF16)
            nc.sync.dma_start(out=wself_t[:], in_=wself[:])
            win_t = cpool.tile([D, D], BF16)
            nc.sync.dma_start(out=win_t[:], in_=win[:])
            wout_t = cpool.tile([D, D], BF16)
            nc.sync.dma_start(out=wout_t[:], in_=wout[:])
            bbc_t = cpool.tile([P, D], BF16)
            nc.sync.dma_start(out=bbc_t[:], in_=bbc[:])
            invc_d_t = cpool.tile([P, NB], F32)
            nc.sync.dma_start(out=invc_d_t[:], in_=invc_d[:])
            invc_s_t = cpool.tile([P, NB], F32)
            nc.sync.dma_start(out=invc_s_t[:], in_=invc_s[:])

            # dram buffers
            cc_in = dram.tile([NBP, D], BF16)
            cc_out = dram.tile([TROWS, D], BF16, addr_space="Shared")
            tout_loc = dram.tile([NBP, D], BF16)

            # preload ALL gather index tiles in one DMA so the prep
            # instructions never wait on index loads
            SW = CH // 16
            sidx_all = cpool.tile([P, NCH * SW], I16)
            nc.sync.dma_start(out=sidx_all[:], in_=gidx[:])
            sidx_tiles = [sidx_all[:, ci * SW : (ci + 1) * SW] for ci in range(NCH)]

            NGRP = CH // P
            NQ = 4
            PREAG = 6  # chunks whose descriptors are emitted during phase A
            gi_tiles = [None] * NCH
            chunk_slot0 = []
            acc = 0
            for n, _hi in chunks:
                chunk_slot0.append(acc)
                acc += n
            gsems = [nc.alloc_semaphore(f"gsem{i}") for i in range(NCH)]

            def prep_chunk(ci):
                n, hi = chunks[ci]
                ngrp = n // P
                gi = gpool.tile([P, NGRP * D], BF16, tag="gi")
                base = HIBASE if hi else 0
                nc.gpsimd.dma_gather(
                    out_ap=gi[:, : ngrp * D].rearrange("p (g d) -> p g d", g=ngrp),
                    in_ap=cc_out[base : base + LOCAP, :],
                    idxs_ap=sidx_tiles[ci][:, : n // 16],
                    num_idxs=n, num_idxs_reg=n, elem_size=D,
                    single_packet=False,
                    prepare_only=True, sem=gsems[ci], queue_num=ci % NQ,
                )
                gi_tiles[ci] = gi

            # ---- Phase A (shared for dst and src passes) ----
            def phase_a(agat, va, KAs, invc_t, w_t, out_dram, add_bias,
                        oh_dram=None):
                j0 = 0
                for b in range(NB):
                    KA = KAs[b]
                    if KA == 0:
                        continue
                    if va is not None:
                        valt = spool.tile([P, KMAX], F32, tag="aval")
                        nc.sync.dma_start(out=valt[:, :KA], in_=va[:, j0 : j0 + KA])
                    gat = pool.tile([P, KMAX * D], BF16, tag="agather")
                    nc.sync.dma_start(
                        out=gat[:, : KA * D], in_=agat[:, j0 * D : (j0 + KA) * D]
                    )
                    if oh_dram is not None:
                        ohblk = pool.tile([P, KMAX * P], BF16, tag="ohd")
                        nc.sync.dma_start(
                            out=ohblk[:, : KA * P],
                            in_=oh_dram[:, j0 * P : (j0 + KA) * P],
                        )
                    ps = psum.tile([P, D], F32, tag="pA")
                    for j in range(KA):
                        if oh_dram is not None:
                            oh = ohblk[:, j * P : (j + 1) * P]
                        else:
                            oht_ = spool.tile([P, P], BF16, tag="oh")
                            nc.vector.tensor_scalar(
                                oht_[:], iota_t[:], valt[:, j : j + 1], None,
                                mybir.AluOpType.is_equal,
                            )
                            oh = oht_[:]
                        nc.tensor.matmul(
                            ps[:], oh, gat[:, j * D : (j + 1) * D],
                            start=(j == 0), stop=(j == KA - 1),
                        )
                    means = spool.tile([P, D], BF16, tag="means")
                    nc.scalar.mul(out=means[:], in_=ps[:], mul=invc_t[:, b : b + 1])
                    pst = psum.tile([P, D], BF16, tag="pB")
                    nc.tensor.transpose(pst[:], means[:], ident[:])
                    meansT = spool.tile([P, D], BF16, tag="meansT")
                    nc.scalar.copy(out=meansT[:], in_=pst[:])
                    psT = psum.tile([P, D], F32, tag="pC")
                    nc.tensor.matmul(psT[:], meansT[:], w_t[:], start=True, stop=True)
                    tt = spool.tile([P, D], BF16, tag="tt")
                    if add_bias:
                        nc.vector.tensor_add(tt[:], psT[:], bbc_t[:])
                    else:
                        nc.scalar.copy(out=tt[:], in_=psT[:])
                    nc.sync.dma_start(out=out_dram[b * P : (b + 1) * P, :], in_=tt[:])
                    j0 += KA

            # emit gather descriptors for the first PREAG chunks during
            # phase A (prepare-only; fired by the post-barrier triggers)
            for ci in range(min(PREAG, NCH)):
                prep_chunk(ci)

            KAd = [KL[b] + KH[b] for b in range(NB)]
            phase_a(agat_d, None, KAd, invc_d_t, win_t, cc_in, True,
                    oh_dram=ohd)
            nc.gpsimd.collective_compute(
                "AllGather", mybir.AluOpType.bypass,
                replica_groups=[list(range(C))],
                ins=[cc_in.opt()], outs=[cc_out.opt()],
            )
            phase_a(agat_s, va_s, KS, invc_s_t, wout_t, tout_loc, False)

            # the all-engine barrier is what orders the triggers after the
            # AllGather (nothing tracks the collective's completion through
            # the prep/trigger path; the barrier + A-src duration covers it)
            tc.strict_bb_all_engine_barrier()
            for q in range(min(NQ, PREAG, NCH)):
                nc.gpsimd.trigger_dma(count=None, queue_num=q)
            next_chunk = min(PREAG, NCH)

            # ---- Phase C ----
            # segment list: (block b, ntiles, xt-slice tile offset) in slot order
            segs = [("lo", b, KL[b]) for b in range(NB) if KL[b] > 0] + [
                ("hi", b, KH[b]) for b in range(NB) if KH[b] > 0
            ]


            slot = 0  # global slot cursor (tiles processed * P)
            t_glob = 0  # global tile cursor
            for kind, b, ntiles in segs:
                # make sure gather chunks covering this segment are issued
                while (next_chunk < NCH
                       and chunk_slot0[next_chunk] < slot + ntiles * P + 2 * CH):
                    ci = next_chunk
                    n, hi = chunks[ci]
                    ngrp = n // P
                    gi = gpool.tile([P, NGRP * D], BF16, tag="gi")
                    base = HIBASE if hi else 0
                    nc.gpsimd.dma_gather(
                        out_ap=gi[:, : ngrp * D].rearrange(
                            "p (g d) -> p g d", g=ngrp),
                        in_ap=cc_out[base : base + LOCAP, :],
                        idxs_ap=sidx_tiles[ci][:, : n // 16],
                        num_idxs=n, num_idxs_reg=n, elem_size=D,
                        single_packet=False,
                    )
                    gi_tiles[ci] = gi
                    next_chunk += 1
                xT = pool.tile([P, KMAX * D], BF16, tag="xT")
                nc.sync.dma_start(
                    out=xT[:, : ntiles * D],
                    in_=agat_dt[:, t_glob * D : (t_glob + ntiles) * D],
                )
                ohT = pool.tile([P, KMAX * P], BF16, tag="ohT")
                nc.sync.dma_start(
                    out=ohT[:, : ntiles * P],
                    in_=ohts[:, t_glob * P : (t_glob + ntiles) * P],
                )
                tout_b = spool.tile([P, D], BF16, tag="toutb")
                nc.sync.dma_start(out=tout_b[:], in_=tout_loc[b * P : (b + 1) * P, :])
                yo = pool.tile([P, KMAX * D], BF16, tag="yo")
                for j in range(ntiles):
                    s = slot + j * P
                    ci = 0
                    while chunk_slot0[ci] + chunks[ci][0] <= s:
                        ci += 1
                    g = (s - chunk_slot0[ci]) // P
                    gi = gi_tiles[ci]
                    psc = psum.tile([P, D], F32, tag="pD")
                    nc.tensor.matmul(
                        psc[:], xT[:, j * D : (j + 1) * D], wself_t[:],
                        start=True, stop=False,
                    )
                    nc.tensor.matmul(
                        psc[:], ohT[:, j * P : (j + 1) * P], tout_b[:],
                        start=False, stop=True,
                    )
                    s2 = spool.tile([P, D], BF16, tag="s2")
                    nc.vector.tensor_add(s2[:], psc[:], gi[:, g * D : (g + 1) * D])
                    t1 = spool.tile([P, D], BF16, tag="t1")
                    nc.scalar.mul(out=t1[:], in_=s2[:], mul=0.2)
                    nc.vector.tensor_max(
                        yo[:, j * D : (j + 1) * D], s2[:], t1[:]
                    )
                nc.sync.dma_start(
                    out=y[:, t_glob * D : (t_glob + ntiles) * D],
                    in_=yo[:, : ntiles * D],
                )
                slot += ntiles * P
                t_glob += ntiles

    nc.compile()
    return nc


def prepare_inputs(cfg, edge_attr, edge_index, W_self_w, W_self_b, W_in_w, W_out_w):
    """Host-side sharding / graph partitioning. Returns (params, in_maps, post)."""
    E, N = cfg["E"], cfg["N"]
    NPC, NB, NBP = _derive(cfg)

    edge_attr = np.asarray(edge_attr, dtype=np.float32)
    src = np.asarray(edge_index[0], dtype=np.int64)
    dst = np.asarray(edge_index[1], dtype=np.int64)

    wself = np.ascontiguousarray(np.asarray(W_self_w, np.float32).T)
    win = np.ascontiguousarray(np.asarray(W_in_w, np.float32).T)
    wout = np.ascontiguousarray(np.asarray(W_out_w, np.float32).T)
    bbc = np.tile(np.asarray(W_self_b, dtype=np.float32)[None, :], (P, 1))
    iota = np.tile(np.arange(P, dtype=np.float32)[None, :], (P, 1))

    src_row = (src // NPC) * NBP + (src % NPC)

    # ---------- phase A-src grouping (per src owner core / block) ----------
    core_s = src // NPC
    loc_s = src - core_s * NPC
    blk_s = loc_s >> 7
    sloc = (loc_s & 127).astype(np.float32)
    key_s = core_s * NB + blk_s
    cnt_s = np.bincount(key_s, minlength=C * NB).reshape(C, NB)
    KS = [int(np.ceil(cnt_s[:, b].max() / P)) for b in range(NB)]
    TOTJS = sum(KS)

    order_s = np.argsort(key_s, kind="stable")
    starts_s = np.zeros(C * NB, dtype=np.int64)
    np.cumsum(cnt_s.ravel()[:-1], out=starts_s[1:])
    j0s = np.zeros(NB, dtype=np.int64)
    np.cumsum(np.asarray(KS[:-1]), out=j0s[1:])
    # slot within core = (j0s[blk] + pos within block) with slot=(j*128+p)
    pos_s = np.arange(E, dtype=np.int64) - starts_s[key_s[order_s]]
    slot_s = j0s[blk_s[order_s]] * P + pos_s  # slot within its core

    # ---------- phase A-dst / phase C grouping ----------
    core_d = dst // NPC
    loc_d = dst - core_d * NPC
    blk_d = loc_d >> 7
    dloc = (loc_d & 127).astype(np.int64)
    key_d = core_d * NB + blk_d
    cnt_d = np.bincount(key_d, minlength=C * NB).reshape(C, NB)
    is_lo_must = src_row < HIBASE
    is_lo_ok = src_row < LOCAP
    cnt_lo_must = np.bincount(key_d, weights=is_lo_must, minlength=C * NB
                              ).reshape(C, NB).astype(np.int64)
    cnt_lo_ok = np.bincount(key_d, weights=is_lo_ok, minlength=C * NB
                            ).reshape(C, NB).astype(np.int64)

    KL, KH = [], []
    for b in range(NB):
        kl_min = int(np.ceil(cnt_lo_must[:, b].max() / P))
        kl_max = int(np.floor(cnt_lo_ok[:, b].min() / P))
        best = None
        for kl in range(kl_min, max(kl_min, kl_max) + 1):
            nhi = np.maximum(cnt_d[:, b] - np.minimum(cnt_lo_ok[:, b], kl * P), 0)
            kh = int(np.ceil(nhi.max() / P))
            if best is None or kl + kh < best[0] + best[1]:
                best = (kl, kh)
        KL.append(best[0])
        KH.append(best[1])
    TOTJ = sum(KL) + sum(KH)
    NLO = sum(KL) * P

    # per-(core, block) edge lists
    order_d = np.argsort(key_d, kind="stable")
    starts_d = np.zeros(C * NB + 1, dtype=np.int64)
    np.cumsum(cnt_d.ravel(), out=starts_d[1:])

    # tile offsets: lo segments (block order), then hi segments
    lo_t0 = np.zeros(NB, dtype=np.int64)
    np.cumsum(np.asarray(KL[:-1]), out=lo_t0[1:])
    hi_t0 = np.zeros(NB, dtype=np.int64)
    np.cumsum(np.asarray(KH[:-1]), out=hi_t0[1:])
    hi_t0 += sum(KL)

    # slot_edge per core: global edge id at each slot, -1 = pad
    TOT_SLOTS = TOTJ * P
    slot_edge = np.full((C, TOT_SLOTS), -1, dtype=np.int64)
    for c in range(C):
        for b in range(NB):
            k = c * NB + b
            e_ids = order_d[starts_d[k] : starts_d[k + 1]]
            if len(e_ids) == 0:
                continue
            lo_mask = is_lo_ok[e_ids]
            lo_ids = e_ids[lo_mask]
            hi_ids = e_ids[~lo_mask]
            cap = KL[b] * P
            if len(lo_ids) > cap:
                # move overlap edges (src_row >= HIBASE) to hi until it fits
                movable = src_row[lo_ids] >= HIBASE
                mv_idx = np.where(movable)[0]
                nmove = len(lo_ids) - cap
                mv = mv_idx[:nmove]
                keep = np.ones(len(lo_ids), dtype=bool)
                keep[mv] = False
                hi_ids = np.concatenate([hi_ids, lo_ids[~keep]])
                lo_ids = lo_ids[keep]
            assert len(hi_ids) <= KH[b] * P, (b, len(hi_ids), KH[b])
            s0 = lo_t0[b] * P
            slot_edge[c, s0 : s0 + len(lo_ids)] = lo_ids
            s0 = hi_t0[b] * P
            slot_edge[c, s0 : s0 + len(hi_ids)] = hi_ids

    # gather chunks
    chunks = []
    off = 0
    NHI = sum(KH) * P
    while off < NLO:
        chunks.append(min(CH, NLO - off))
        off += CH
    off = 0
    while off < NHI:
        chunks.append(min(CH, NHI - off))
        off += CH
    NCH = len(chunks)

    def wrap_idx(vals):
        n = len(vals)
        t = np.zeros((16, CH // 16), dtype=np.int16)
        t[np.arange(n) % 16, np.arange(n) // 16] = vals.astype(np.int16)
        return np.tile(t, (8, 1))

    # phase A-dst uses its own block-contiguous tile order (lo+hi per block);
    # phase C arrays stay in slot order ([all lo][all hi]).
    tile_perm = np.concatenate(
        [
            np.concatenate([
                np.arange(lo_t0[b], lo_t0[b] + KL[b]),
                np.arange(hi_t0[b], hi_t0[b] + KH[b]),
            ])
            for b in range(NB)
        ]
    ).astype(np.int64)

    in_maps = []
    for c in range(C):
        se = slot_edge[c]
        valid = se >= 0
        ge = np.where(valid, se, 0)

        xs = np.where(valid[:, None], edge_attr[ge], 0).astype(np.float32)
        xs3 = xs.reshape(TOTJ, P, D)
        agat_c = np.ascontiguousarray(
            xs3[tile_perm].transpose(1, 0, 2).reshape(P, TOTJ * D)).astype(BF16NP)
        agat_ct = np.ascontiguousarray(
            xs3.transpose(2, 0, 1).reshape(P, TOTJ * P)).astype(BF16NP)

        dv = np.where(valid, dloc[ge], -1)
        oht = np.zeros((TOTJ, P, P), dtype=np.float32)
        sl = np.arange(TOT_SLOTS)[valid]
        oht[sl // P, dv[valid], sl % P] = 1.0
        oht_c = np.ascontiguousarray(
            oht.transpose(1, 0, 2).reshape(P, TOTJ * P)).astype(BF16NP)
        ohd_c = np.ascontiguousarray(
            oht[tile_perm].transpose(2, 0, 1).reshape(P, TOTJ * P)).astype(BF16NP)

        # gather indices
        gidx_full = np.where(valid, src_row[ge], 0)
        gidx_full[NLO:] = np.where(valid[NLO:], gidx_full[NLO:] - HIBASE, 0)
        assert gidx_full.min() >= 0 and gidx_full.max() < LOCAP
        gx = np.zeros((NCH, P, CH // 16), dtype=np.int16)
        off = 0
        for ci, n in enumerate(chunks):
            gx[ci] = wrap_idx(gidx_full[off : off + n])
            off += n
        gx = np.ascontiguousarray(
            gx.transpose(1, 0, 2).reshape(P, NCH * (CH // 16)))

        # phase A-src arrays
        m = core_s[order_s] == c
        sslot = slot_s[m]
        sids = order_s[m]
        xs_s = np.zeros((TOTJS * P, D), dtype=np.float32)
        xs_s[sslot] = edge_attr[sids]
        va_sf = np.full(TOTJS * P, -1.0, dtype=np.float32)
        va_sf[sslot] = sloc[sids]
        agat_s_c = np.ascontiguousarray(
            xs_s.reshape(TOTJS, P, D).transpose(1, 0, 2).reshape(P, TOTJS * D)
        ).astype(BF16NP)
        va_s_c = np.ascontiguousarray(va_sf.reshape(TOTJS, P).T)

        # inverse counts
        def build_inv(node_of_edge):
            cnt = np.bincount(node_of_edge, minlength=N).astype(np.float32)
            inv = 1.0 / np.maximum(cnt, 1.0)
            pad = np.zeros(NBP, dtype=np.float32)
            pad[:NPC] = inv[c * NPC : (c + 1) * NPC]
            return np.ascontiguousarray(pad.reshape(NB, P).T)

        in_maps.append(
            dict(
                agat_d=agat_c, agat_dt=agat_ct, ohts=oht_c, ohd=ohd_c,
                agat_s=agat_s_c, va_s=va_s_c,
                invc_d=build_inv(dst), invc_s=build_inv(src),
                gidx=gx,
                wself=wself.astype(BF16NP), win=win.astype(BF16NP),
                wout=wout.astype(BF16NP), bbc=bbc.astype(BF16NP),
                iota=iota,
            )
        )

    slot_edge_all = slot_edge

    def postprocess(results):
        full = np.empty((E, D), dtype=np.float32)
        for c in range(C):
            yv = results[c]["y"].astype(np.float32)
            yv = yv.reshape(P, TOTJ, D).transpose(1, 0, 2).reshape(TOT_SLOTS, D)
            se = slot_edge_all[c]
            valid = se >= 0
            full[se[valid]] = yv[valid]
        return full

    params = (tuple(KL), tuple(KH), tuple(KS))
    return params, in_maps, postprocess


_NC_CACHE = {}


def run(cfg, inputs, trace=False, trace_kwargs=None):
    params, in_maps, post = prepare_inputs(
        cfg,
        inputs["edge_attr"],
        inputs["edge_index"],
        inputs["W_self_w"],
        inputs["W_self_b"],
        inputs["W_in_w"],
        inputs["W_out_w"],
    )
    key = (tuple(sorted(cfg.items())), params)
    if key not in _NC_CACHE:
        _NC_CACHE[key] = build_kernel(cfg, list(params[0]), list(params[1]),
                                      list(params[2]))
    nc = _NC_CACHE[key]
    kw = {}
    if trace:
        kw["trace"] = True
        if trace_kwargs:
            kw.update(trace_kwargs)
    res = run_bass_kernel_spmd(nc, in_maps, core_ids=list(range(C)), **kw)
    return post(res.results), res


def kernel(**inputs) -> np.ndarray:
    out, _ = run(_cfg_full(), inputs)
    return out.astype(np.float32)
